# revision 26
# baseline (speedup 1.0000x reference)
"""Distributed CAP-memory loss kernel for 8 TRN2 NeuronCores.

Problem (see reference): given unit-norm features [B=256, D=2048] and a
memory bank [6, 2000, 2048], compute
  loss = sum_cam mean_cam(per-camera proxy CE)
       + 0.5 * sum_cam mean_cam(assoc loss over 6 positives + 50 hard negatives)

Distribution strategy (column/class sharding, interleaved):
  The 12000 memory rows are split so core k owns columns
  {j*2000 + k*250 + r : j in [0,6), r in [0,250)} -- i.e. an identical
  250-wide slice of every camera block.  All 8 cores therefore run the
  exact same program (true SPMD) on 1500 columns each:
    * sims_local = feats @ memT_local            (PE, f32[r])
    * per-camera-block partial sum(exp(20*sims)) (ACT, accum)
    * positive extraction via iota==label mask   (DVE)
    * top-16 of each positive-masked 500-col     (DVE max8 + match_replace)
      chunk -> 48 candidates per core, 384 per row globally
  The host merges the per-core stats ([256, 60] each): global top-50
  from the 384 candidates (with an exactness certificate -- see
  host_combine -- and an exact fallback), log-sum-exp combines, segment
  sums -> scalar loss.

Inputs are pre-transposed on the host so every matmul operand is
K(=D)-major and no on-device transpose is needed.
"""

import os
import sys
import types

import numpy as np

# ---------------------------------------------------------------- constants
B = 256          # batch
D = 2048         # feature dim
NCAMS = 6
C = 2000         # classes per camera
NG = NCAMS * C   # 12000 global columns
M = 8            # cores
W = C // M       # 250: per-core slice width inside each camera block
NL = NCAMS * W   # 1500 local columns per core
P = 128          # partitions
KO = D // P      # 16 contraction chunks
BT = B // P      # 2 batch tiles
NT = 3           # matmul column chunks
NCHUNK = NL // NT  # 500
BETA = 0.05
INV_BETA = 1.0 / BETA  # 20.0
BG_KNN = 50
KITC = 2         # top-8 iterations per 500-col chunk -> 16 cand/chunk
NCAND = KITC * 8 * NT  # 48 candidates per core
# Mask value -4: positives become sims-4 in [-5,-3], below every genuine
# cosine sim (>= -1), so they never reach a top-16.  Multiplying the masked
# value by -4 (exact in fp) gives 16 - 4*sims at the positive and 0
# elsewhere, so the host recovers the positive sim as (16 - sum)/4.
MASK_VAL = -4.0
REPL_VAL = -30000.0
OUTC = NCAND + 2 * NCAMS  # 48 topk | 6 pos | 6 sumexp

LAST_EXEC_NS = None
FALLBACK_COUNT = 0
_NC_CACHE = {}


def _install_axon_ntff_hook():
    """The agent image's antenv lacks axon_hooks; synthesize it so
    run_bass_kernel_spmd(trace=True) can capture NTFF profiles."""
    if "antenv.axon_hooks" in sys.modules:
        return
    mod = types.ModuleType("antenv.axon_hooks")
    state = {"hook": None}
    mod.set_axon_ntff_profile_hook = lambda h: state.__setitem__("hook", h)
    mod.get_axon_ntff_profile_hook = lambda: state["hook"]
    sys.modules["antenv.axon_hooks"] = mod
    try:
        import antenv

        antenv.axon_hooks = mod
    except Exception:
        pass
    try:
        from trn_agent_boot.trn_boot import _ntff_profile_via_ctypes

        hook = _ntff_profile_via_ctypes("/opt/axon/libaxon_pjrt.so")
        if hook is not None:
            mod.set_axon_ntff_profile_hook(hook)
    except Exception:
        pass


def build_nc(mm_dtype_name: str = "float32r"):
    """Build + compile the single SPMD Bass program shared by all 8 cores."""
    import concourse.bacc as bacc
    import concourse.mybir as mybir
    import concourse.tile as tile

    f32 = mybir.dt.float32
    mm_dt = getattr(mybir.dt, mm_dtype_name)
    A = mybir.AluOpType
    AF = mybir.ActivationFunctionType

    nc = bacc.Bacc(
        "TRN2",
        target_bir_lowering=False,
        debug=False,
        enable_asserts=False,
        num_devices=M,
    )

    # Host-packed layouts: one fully-contiguous row per SBUF partition so
    # DMA descriptors are 8KB/12KB (>=4KB saturates the HBM bus width).
    featsT_d = nc.dram_tensor("featsT", [P, KO * B], mm_dt, kind="ExternalInput")
    memT_d = nc.dram_tensor("memT", [P, KO * NL], mm_dt, kind="ExternalInput")
    iota_d = nc.dram_tensor("iotaF", [P, NL], f32, kind="ExternalInput")
    lab_d = nc.dram_tensor("labF", [B, 1], f32, kind="ExternalInput")
    out_d = nc.dram_tensor("out", [B, OUTC], f32, kind="ExternalOutput")

    lab_r = lab_d.rearrange("(bt p) o -> p (bt o)", p=P)
    KC = 4  # memT arrives in 4 ko-chunks; PSUM accumulates partial-K
    KPC = KO // KC

    with tile.TileContext(nc) as tc:
        with (
            tc.tile_pool(name="big", bufs=1) as big,
            tc.tile_pool(name="work", bufs=BT) as work,
            tc.tile_pool(name="scr", bufs=4) as scr,
            tc.tile_pool(name="psum", bufs=BT * NT, space="PSUM") as psum,
        ):
            # Spread the input DMAs over three engine queues so the first
            # matmul chunk isn't serialized behind the small aux loads.
            featsT_sb = big.tile([P, KO * B], mm_dt)
            nc.scalar.dma_start(featsT_sb[:], featsT_d[:])
            memT_sb = big.tile([P, KO * NL], mm_dt)
            kcq = [nc.sync, nc.scalar, nc.gpsimd, nc.sync]
            for kc in range(KC):
                ksl = slice(kc * KPC * NL, (kc + 1) * KPC * NL)
                kcq[kc].dma_start(memT_sb[:, ksl], memT_d[:, ksl])
            iota_sb = big.tile([P, NL], f32)
            nc.gpsimd.dma_start(iota_sb[:], iota_d[:])
            lab_sb = big.tile([P, BT], f32)
            nc.gpsimd.dma_start(lab_sb[:], lab_r)

            pm = [
                work.tile([P, NL], f32, tag="pm", name=f"pm{b}")
                for b in range(BT)
            ]
            masked = [
                work.tile([P, NL], f32, tag="masked", name=f"masked{b}")
                for b in range(BT)
            ]
            outs = [
                work.tile([P, OUTC], f32, tag="outs", name=f"outs{b}")
                for b in range(BT)
            ]
            tmp = [
                work.tile([P, NL], f32, tag="tmp", name=f"tmp{b}")
                for b in range(BT)
            ]

            # positive mask: -20000 where (global_col % 2000) == label, else 0
            for bt in range(BT):
                nc.vector.tensor_scalar(
                    pm[bt][:],
                    iota_sb[:],
                    lab_sb[:, bt : bt + 1],
                    MASK_VAL,
                    op0=A.is_equal,
                    op1=A.mult,
                )

            pstiles = [
                psum.tile([P, NCHUNK], f32, tag="ps", name=f"ps{b}_{n}")
                for b in range(BT)
                for n in range(NT)
            ]

            def mm(kc, bt, nt, kof):
                ko = kc * KPC + kof
                nc.tensor.matmul(
                    pstiles[bt * NT + nt][:],
                    featsT_sb[:, ko * B + bt * P : ko * B + (bt + 1) * P],
                    memT_sb[:, ko * NL + nt * NCHUNK : ko * NL + (nt + 1) * NCHUNK],
                    start=(ko == 0),
                    stop=(ko == KO - 1),
                )

            def epilogue(bt, nt):
                ps = pstiles[bt * NT + nt]
                nsl = slice(nt * NCHUNK, (nt + 1) * NCHUNK)
                # masked sims in one pass: sims + pm
                nc.vector.tensor_tensor(
                    out=masked[bt][:, nsl],
                    in0=ps[:],
                    in1=pm[bt][:, nsl],
                    op=A.add,
                )
                for s in range(NCHUNK // W):
                    j = nt * (NCHUNK // W) + s
                    pse = ps[:, s * W : (s + 1) * W]
                    # per-camera-block sum(exp(sims/beta)); sims in (-1,1)
                    # so exp(20*sims) stays in f32 range without bias
                    et = scr.tile([P, W], f32, tag="exp")
                    nc.scalar.activation(
                        et[:],
                        pse,
                        AF.Exp,
                        scale=INV_BETA,
                        accum_out=outs[bt][
                            :, NCAND + NCAMS + j : NCAND + NCAMS + j + 1
                        ],
                    )
                # positive extraction: (sims+pm)*pm = 16-4*sims at the
                # positive slot, 0 elsewhere (gpsimd: SBUF-only inputs)
                nc.gpsimd.tensor_tensor(
                    out=tmp[bt][:, nsl],
                    in0=masked[bt][:, nsl],
                    in1=pm[bt][:, nsl],
                    op=A.mult,
                )
                nc.vector.reduce_sum(
                    out=outs[bt][:, NCAND + 2 * nt : NCAND + 2 * nt + 2],
                    in_=tmp[bt][:, nsl].rearrange("p (j w) -> p j w", w=W),
                    axis=mybir.AxisListType.X,
                )
                # top-16 of this 500-col chunk
                for it in range(KITC):
                    col = (nt * KITC + it) * 8
                    nc.vector.max(
                        out=outs[bt][:, col : col + 8],
                        in_=masked[bt][:, nsl],
                    )
                    if it < KITC - 1:
                        nc.vector.match_replace(
                            out=masked[bt][:, nsl],
                            in_to_replace=outs[bt][:, col : col + 8],
                            in_values=masked[bt][:, nsl],
                            imm_value=REPL_VAL,
                        )

            for kc in range(KC):
                for bt in range(BT):
                    if kc < KC - 1:
                        for kof in range(KPC):
                            for nt in range(NT):
                                mm(kc, bt, nt, kof)
                    else:
                        for nt in range(NT):
                            for kof in range(KPC):
                                mm(kc, bt, nt, kof)
                            epilogue(bt, nt)

            for bt in range(BT):
                nc.sync.dma_start(out_d[bt * P : (bt + 1) * P, :], outs[bt][:])

    nc.compile()
    return nc


def get_nc(mm_dtype_name: str = None):
    if mm_dtype_name is None:
        mm_dtype_name = os.environ.get("CAP_MM_DTYPE", "bfloat16")
    if mm_dtype_name not in _NC_CACHE:
        _NC_CACHE[mm_dtype_name] = build_nc(mm_dtype_name)
    return _NC_CACHE[mm_dtype_name]


def shard_cols(k: int) -> np.ndarray:
    """Global memory-bank columns owned by core k."""
    return (
        np.arange(NCAMS)[:, None] * C + k * W + np.arange(W)[None, :]
    ).reshape(-1)


def _mm_np_dtype():
    name = os.environ.get("CAP_MM_DTYPE", "bfloat16")
    if name == "bfloat16":
        import ml_dtypes

        return np.dtype(ml_dtypes.bfloat16)
    return np.dtype(np.float32)


def pack_featsT(features: np.ndarray) -> np.ndarray:
    """[B, D] -> [P, KO*B] with row p holding feats.T[ko*128+p, :] runs."""
    arr = features.T.reshape(KO, P, B).transpose(1, 0, 2).reshape(P, KO * B)
    return np.ascontiguousarray(arr).astype(_mm_np_dtype())


def pack_memT(mem_flat: np.ndarray, cols: np.ndarray) -> np.ndarray:
    """[NG, D] -> [P, KO*NL] packed like pack_featsT for this core's cols."""
    arr = (
        mem_flat[cols].T.reshape(KO, P, NL).transpose(1, 0, 2).reshape(P, KO * NL)
    )
    return np.ascontiguousarray(arr).astype(_mm_np_dtype())


def make_in_maps(features: np.ndarray, labels: np.ndarray):
    featsT = pack_featsT(features)
    labF = labels.astype(np.float32).reshape(B, 1)
    return featsT, labF


def _loss_from_parts(pos_vals, lse_block, top50, cams):
    rows = np.arange(B)
    ce = lse_block[rows, cams] - INV_BETA * pos_vals[rows, cams]
    logits = np.concatenate([INV_BETA * pos_vals, INV_BETA * top50], axis=1)
    mx = logits.max(axis=1, keepdims=True)
    lse56 = mx[:, 0] + np.log(np.exp(logits - mx).sum(axis=1))
    assoc = lse56 - (INV_BETA / NCAMS) * pos_vals.sum(axis=1)

    counts = np.bincount(cams, minlength=NCAMS).astype(np.float64)
    ce_sum = np.bincount(cams, weights=ce, minlength=NCAMS)
    as_sum = np.bincount(cams, weights=assoc, minlength=NCAMS)
    safe = np.maximum(counts, 1.0)
    present = counts > 0
    return np.sum(np.where(present, ce_sum / safe, 0.0)) + np.sum(
        np.where(present, 0.5 * as_sum / safe, 0.0)
    )


def host_combine(outs, cams, features=None, memory=None, labels=None):
    """outs: [M, B, OUTC] device results; cams: [B] int."""
    global FALLBACK_COUNT
    cand = outs[:, :, :NCAND].astype(np.float64)  # [M, B, 48]
    posp = outs[:, :, NCAND : NCAND + NCAMS].astype(np.float64)
    sexp = outs[:, :, NCAND + NCAMS :].astype(np.float64)

    # device stores sum((sims+pm)*pm) = 16 - 4*sims_pos per (core, block)
    pos_vals = (MASK_VAL * MASK_VAL - posp.sum(axis=0)) / (-MASK_VAL)
    s_block = sexp.sum(axis=0)  # [B, 6] sum(exp(20*sims)) per camera block
    lse_block = np.log(s_block)  # logsumexp of own-camera logits

    # [B, M*NT, 16] per-(core,chunk) candidate lists
    percl = (
        cand.transpose(1, 0, 2)
        .reshape(B, M, NT, KITC * 8)
        .reshape(B, M * NT, KITC * 8)
    )
    flat = percl.reshape(B, -1)
    top50 = -np.partition(-flat, BG_KNN - 1, axis=1)[:, :BG_KNN]
    t50 = top50[:, BG_KNN - 1]  # [B] 50th largest of the union

    # Exactness certificate: every (core,chunk)'s smallest extracted
    # candidate must be strictly below the union's 50th value, which
    # proves no unseen value can reach the global top-50.
    cmin = percl.min(axis=2)  # [B, M*NT]
    bad = (cmin >= t50[:, None]).any(axis=1)
    if bad.any():
        # Exact fallback for the (astronomically unlikely) insufficient
        # rows: recompute their full similarity row on the host.
        FALLBACK_COUNT += int(bad.sum())
        assert features is not None and memory is not None
        mem_flat = np.asarray(memory, np.float32).reshape(NG, D)
        lab = np.asarray(labels).astype(np.int64)
        idx = np.nonzero(bad)[0]
        sims = (
            np.asarray(features, np.float32)[idx] @ mem_flat.T
        )  # [nbad, NG]
        cols = np.arange(NG)
        for pos, i in enumerate(idx):
            row = sims[pos].copy()
            row[cols % C == lab[i]] = MASK_VAL
            top50[i] = -np.sort(-row)[:BG_KNN]

    return np.float32(_loss_from_parts(pos_vals, lse_block, top50, cams))


def kernel(features, memory, cams, labels, trace: bool = None):
    global LAST_EXEC_NS
    _install_axon_ntff_hook()
    from concourse.bass_utils import run_bass_kernel_spmd

    features = np.asarray(features, dtype=np.float32)
    memory = np.asarray(memory, dtype=np.float32)
    cams = np.asarray(cams).astype(np.int64)
    labels = np.asarray(labels).astype(np.int64)

    nc = get_nc()

    mem_flat = memory.reshape(NG, D)
    featsT, labF = make_in_maps(features, labels)
    in_maps = []
    for k in range(M):
        cols = shard_cols(k)
        memT = pack_memT(mem_flat, cols)
        iotaF = np.broadcast_to(
            (cols % C).astype(np.float32), (P, NL)
        ).copy()
        in_maps.append(
            {"featsT": featsT, "memT": memT, "iotaF": iotaF, "labF": labF}
        )

    if trace is None:
        trace = os.environ.get("CAP_TRACE", "1") == "1"
    res = run_bass_kernel_spmd(
        nc, in_maps, core_ids=list(range(M)), trace=trace
    )
    if res.exec_time_ns is not None:
        LAST_EXEC_NS = res.exec_time_ns

    outs = np.stack([r["out"] for r in res.results])  # [M, B, OUTC]
    return np.asarray(
        host_combine(outs, cams, features, memory, labels), dtype=np.float32
    )


# ------------------------------------------------------------------ helpers
def expected_core_out(features, memory, labels, k: int) -> np.ndarray:
    """Numpy model of what core k's device program should output [B, OUTC]."""
    mem_flat = np.asarray(memory, np.float32).reshape(NG, D)
    cols = shard_cols(k)
    sims = np.asarray(features, np.float32) @ mem_flat[cols].T  # [B, NL]
    lab = np.asarray(labels).astype(np.int64)
    pmask = (cols % C)[None, :] == lab[:, None]  # [B, NL]
    out = np.zeros((B, OUTC), np.float32)
    maskedv = sims + (pmask * np.float32(MASK_VAL)).astype(np.float32)
    for j in range(NCAMS):
        jsl = slice(j * W, (j + 1) * W)
        out[:, NCAND + j] = (
            maskedv[:, jsl] * (pmask[:, jsl] * np.float32(MASK_VAL))
        ).sum(axis=1)
        out[:, NCAND + NCAMS + j] = np.exp(
            INV_BETA * sims[:, jsl].astype(np.float64)
        ).sum(axis=1)
    for nt in range(NT):
        chunk = maskedv[:, nt * NCHUNK : (nt + 1) * NCHUNK]
        srt = -np.sort(-chunk, axis=1)
        out[:, nt * KITC * 8 : (nt + 1) * KITC * 8] = srt[:, : KITC * 8]
    return out


# revision 33
# speedup vs baseline: 1.0598x; 1.0598x over previous
"""Distributed CAP-memory loss kernel for 8 TRN2 NeuronCores.

Problem (see reference): given unit-norm features [B=256, D=2048] and a
memory bank [6, 2000, 2048], compute
  loss = sum_cam mean_cam(per-camera proxy CE)
       + 0.5 * sum_cam mean_cam(assoc loss over 6 positives + 50 hard negatives)

Distribution strategy (column/class sharding, interleaved):
  The 12000 memory rows are split so core k owns columns
  {j*2000 + k*250 + r : j in [0,6), r in [0,250)} -- i.e. an identical
  250-wide slice of every camera block.  All 8 cores therefore run the
  exact same program (true SPMD) on 1500 columns each:
    * sims_local = feats @ memT_local            (PE, f32[r])
    * per-camera-block partial sum(exp(20*sims)) (ACT, accum)
    * positive extraction via iota==label mask   (DVE)
    * top-16 of each positive-masked 500-col     (DVE max8 + match_replace)
      chunk -> 48 candidates per core, 384 per row globally
  The host merges the per-core stats ([256, 60] each): global top-50
  from the 384 candidates (with an exactness certificate -- see
  host_combine -- and an exact fallback), log-sum-exp combines, segment
  sums -> scalar loss.

Inputs are pre-transposed on the host so every matmul operand is
K(=D)-major and no on-device transpose is needed.
"""

import os
import sys
import types

import numpy as np

# ---------------------------------------------------------------- constants
B = 256          # batch
D = 2048         # feature dim
NCAMS = 6
C = 2000         # classes per camera
NG = NCAMS * C   # 12000 global columns
M = 8            # cores
W = C // M       # 250: per-core slice width inside each camera block
NL = NCAMS * W   # 1500 local columns per core
P = 128          # partitions
KO = D // P      # 16 contraction chunks
BT = B // P      # 2 batch tiles
NT = 3           # matmul column chunks
NCHUNK = NL // NT  # 500
BETA = 0.05
INV_BETA = 1.0 / BETA  # 20.0
BG_KNN = 50
KITC = 2         # top-8 iterations per 500-col chunk -> 16 cand/chunk
NCAND = KITC * 8 * NT  # 48 candidates per core
# Mask value -4: positives become sims-4 in [-5,-3], below every genuine
# cosine sim (>= -1), so they never reach a top-16.  Multiplying the masked
# value by -4 (exact in fp) gives 16 - 4*sims at the positive and 0
# elsewhere, so the host recovers the positive sim as (16 - sum)/4.
MASK_VAL = -4.0
REPL_VAL = -30000.0
OUTC = NCAND + 2 * NCAMS  # 48 topk | 6 pos | 6 sumexp

LAST_EXEC_NS = None
FALLBACK_COUNT = 0
_NC_CACHE = {}


def _install_axon_ntff_hook():
    """The agent image's antenv lacks axon_hooks; synthesize it so
    run_bass_kernel_spmd(trace=True) can capture NTFF profiles."""
    if "antenv.axon_hooks" in sys.modules:
        return
    mod = types.ModuleType("antenv.axon_hooks")
    state = {"hook": None}
    mod.set_axon_ntff_profile_hook = lambda h: state.__setitem__("hook", h)
    mod.get_axon_ntff_profile_hook = lambda: state["hook"]
    sys.modules["antenv.axon_hooks"] = mod
    try:
        import antenv

        antenv.axon_hooks = mod
    except Exception:
        pass
    try:
        from trn_agent_boot.trn_boot import _ntff_profile_via_ctypes

        hook = _ntff_profile_via_ctypes("/opt/axon/libaxon_pjrt.so")
        if hook is not None:
            mod.set_axon_ntff_profile_hook(hook)
    except Exception:
        pass


def build_nc(mm_dtype_name: str = "float32r"):
    """Build + compile the single SPMD Bass program shared by all 8 cores."""
    import concourse.bacc as bacc
    import concourse.mybir as mybir
    import concourse.tile as tile

    f32 = mybir.dt.float32
    mm_dt = getattr(mybir.dt, mm_dtype_name)
    A = mybir.AluOpType
    AF = mybir.ActivationFunctionType

    nc = bacc.Bacc(
        "TRN2",
        target_bir_lowering=False,
        debug=False,
        enable_asserts=False,
        num_devices=M,
    )

    # Host-packed layouts: one fully-contiguous row per SBUF partition.
    featsT_d = nc.dram_tensor("featsT", [P, KO * B], mm_dt, kind="ExternalInput")
    memT_d = nc.dram_tensor("memT", [P, KO * NL], mm_dt, kind="ExternalInput")
    # labAdj = label - 250*core_id: the on-device iota holds (col % 250), so
    # equality against labAdj marks exactly this core's positive columns.
    lab_d = nc.dram_tensor("labF", [B, 1], f32, kind="ExternalInput")
    out_d = nc.dram_tensor("out", [B, OUTC], f32, kind="ExternalOutput")

    lab_r = lab_d.rearrange("(bt p) o -> p (bt o)", p=P)

    with tile.TileContext(nc) as tc:
        with (
            tc.tile_pool(name="big", bufs=1) as big,
            tc.tile_pool(name="work", bufs=BT) as work,
            tc.tile_pool(name="scr", bufs=4) as scr,
            tc.tile_pool(name="psum", bufs=BT * NT, space="PSUM") as psum,
        ):
            # Stream inputs in PE-consumption order (ko-major), round-robin
            # across the three DMA-capable queues (~130 GB/s each, ~360
            # aggregate) so the matmul pipeline starts within a few us.
            featsT_sb = big.tile([P, KO * B], mm_dt)
            memT_sb = big.tile([P, KO * NL], mm_dt)
            queues = [nc.sync, nc.scalar, nc.gpsimd]
            for ko in range(KO):
                fsl = slice(ko * B, (ko + 1) * B)
                queues[ko % 3].dma_start(featsT_sb[:, fsl], featsT_d[:, fsl])
                for nt in range(NT):
                    msl = slice(
                        ko * NL + nt * NCHUNK, ko * NL + (nt + 1) * NCHUNK
                    )
                    queues[nt].dma_start(memT_sb[:, msl], memT_d[:, msl])
            lab_sb = big.tile([P, BT], f32)
            nc.gpsimd.dma_start(lab_sb[:], lab_r)
            iota_sb = big.tile([P, NL], f32)
            nc.gpsimd.iota(
                iota_sb[:].rearrange("p (j w) -> p j w", w=W),
                pattern=[[0, NCAMS], [1, W]],
                base=0,
                channel_multiplier=0,
                allow_small_or_imprecise_dtypes=True,
            )

            pm = [
                work.tile([P, NL], f32, tag="pm", name=f"pm{b}")
                for b in range(BT)
            ]
            masked = [
                work.tile([P, NL], f32, tag="masked", name=f"masked{b}")
                for b in range(BT)
            ]
            outs = [
                work.tile([P, OUTC], f32, tag="outs", name=f"outs{b}")
                for b in range(BT)
            ]
            tmp = [
                work.tile([P, NL], f32, tag="tmp", name=f"tmp{b}")
                for b in range(BT)
            ]

            # positive mask: -20000 where (global_col % 2000) == label, else 0
            for bt in range(BT):
                nc.vector.tensor_scalar(
                    pm[bt][:],
                    iota_sb[:],
                    lab_sb[:, bt : bt + 1],
                    MASK_VAL,
                    op0=A.is_equal,
                    op1=A.mult,
                )

            pstiles = [
                psum.tile([P, NCHUNK], f32, tag="ps", name=f"ps{b}_{n}")
                for b in range(BT)
                for n in range(NT)
            ]

            def mm(bt, nt, ko):
                nc.tensor.matmul(
                    pstiles[bt * NT + nt][:],
                    featsT_sb[:, ko * B + bt * P : ko * B + (bt + 1) * P],
                    memT_sb[:, ko * NL + nt * NCHUNK : ko * NL + (nt + 1) * NCHUNK],
                    start=(ko == 0),
                    stop=(ko == KO - 1),
                )

            def epilogue(bt, nt):
                ps = pstiles[bt * NT + nt]
                nsl = slice(nt * NCHUNK, (nt + 1) * NCHUNK)
                # masked sims in one pass: sims + pm
                nc.vector.tensor_tensor(
                    out=masked[bt][:, nsl],
                    in0=ps[:],
                    in1=pm[bt][:, nsl],
                    op=A.add,
                )
                for s in range(NCHUNK // W):
                    j = nt * (NCHUNK // W) + s
                    pse = ps[:, s * W : (s + 1) * W]
                    # per-camera-block sum(exp(sims/beta)); sims in (-1,1)
                    # so exp(20*sims) stays in f32 range without bias
                    et = scr.tile([P, W], f32, tag="exp")
                    nc.scalar.activation(
                        et[:],
                        pse,
                        AF.Exp,
                        scale=INV_BETA,
                        accum_out=outs[bt][
                            :, NCAND + NCAMS + j : NCAND + NCAMS + j + 1
                        ],
                    )
                # positive extraction: (sims+pm)*pm = 16-4*sims at the
                # positive slot, 0 elsewhere (gpsimd: SBUF-only inputs)
                nc.gpsimd.tensor_tensor(
                    out=tmp[bt][:, nsl],
                    in0=masked[bt][:, nsl],
                    in1=pm[bt][:, nsl],
                    op=A.mult,
                )
                nc.vector.reduce_sum(
                    out=outs[bt][:, NCAND + 2 * nt : NCAND + 2 * nt + 2],
                    in_=tmp[bt][:, nsl].rearrange("p (j w) -> p j w", w=W),
                    axis=mybir.AxisListType.X,
                )
                # top-16 of this 500-col chunk
                for it in range(KITC):
                    col = (nt * KITC + it) * 8
                    nc.vector.max(
                        out=outs[bt][:, col : col + 8],
                        in_=masked[bt][:, nsl],
                    )
                    if it < KITC - 1:
                        nc.vector.match_replace(
                            out=masked[bt][:, nsl],
                            in_to_replace=outs[bt][:, col : col + 8],
                            in_values=masked[bt][:, nsl],
                            imm_value=REPL_VAL,
                        )

            for ko in range(KO - 1):
                for bt in range(BT):
                    for nt in range(NT):
                        mm(bt, nt, ko)
            for bt in range(BT):
                for nt in range(NT):
                    mm(bt, nt, KO - 1)
                    epilogue(bt, nt)

            for bt in range(BT):
                nc.sync.dma_start(out_d[bt * P : (bt + 1) * P, :], outs[bt][:])

    nc.compile()
    return nc


def get_nc(mm_dtype_name: str = None):
    if mm_dtype_name is None:
        mm_dtype_name = os.environ.get("CAP_MM_DTYPE", "bfloat16")
    if mm_dtype_name not in _NC_CACHE:
        _NC_CACHE[mm_dtype_name] = build_nc(mm_dtype_name)
    return _NC_CACHE[mm_dtype_name]


def shard_cols(k: int) -> np.ndarray:
    """Global memory-bank columns owned by core k."""
    return (
        np.arange(NCAMS)[:, None] * C + k * W + np.arange(W)[None, :]
    ).reshape(-1)


def _mm_np_dtype():
    name = os.environ.get("CAP_MM_DTYPE", "bfloat16")
    if name == "bfloat16":
        import ml_dtypes

        return np.dtype(ml_dtypes.bfloat16)
    return np.dtype(np.float32)


def pack_featsT(features: np.ndarray) -> np.ndarray:
    """[B, D] -> [P, KO*B] with row p holding feats.T[ko*128+p, :] runs."""
    arr = features.T.reshape(KO, P, B).transpose(1, 0, 2).reshape(P, KO * B)
    return np.ascontiguousarray(arr).astype(_mm_np_dtype())


def pack_memT(mem_flat: np.ndarray, cols: np.ndarray) -> np.ndarray:
    """[NG, D] -> [P, KO*NL] packed like pack_featsT for this core's cols."""
    arr = (
        mem_flat[cols].T.reshape(KO, P, NL).transpose(1, 0, 2).reshape(P, KO * NL)
    )
    return np.ascontiguousarray(arr).astype(_mm_np_dtype())


def make_in_maps(features: np.ndarray, labels: np.ndarray):
    featsT = pack_featsT(features)
    return featsT


def lab_adj(labels: np.ndarray, k: int) -> np.ndarray:
    return (labels.astype(np.float32) - np.float32(W * k)).reshape(B, 1)


def _loss_from_parts(pos_vals, lse_block, top50, cams):
    rows = np.arange(B)
    ce = lse_block[rows, cams] - INV_BETA * pos_vals[rows, cams]
    logits = np.concatenate([INV_BETA * pos_vals, INV_BETA * top50], axis=1)
    mx = logits.max(axis=1, keepdims=True)
    lse56 = mx[:, 0] + np.log(np.exp(logits - mx).sum(axis=1))
    assoc = lse56 - (INV_BETA / NCAMS) * pos_vals.sum(axis=1)

    counts = np.bincount(cams, minlength=NCAMS).astype(np.float64)
    ce_sum = np.bincount(cams, weights=ce, minlength=NCAMS)
    as_sum = np.bincount(cams, weights=assoc, minlength=NCAMS)
    safe = np.maximum(counts, 1.0)
    present = counts > 0
    return np.sum(np.where(present, ce_sum / safe, 0.0)) + np.sum(
        np.where(present, 0.5 * as_sum / safe, 0.0)
    )


def host_combine(outs, cams, features=None, memory=None, labels=None):
    """outs: [M, B, OUTC] device results; cams: [B] int."""
    global FALLBACK_COUNT
    cand = outs[:, :, :NCAND].astype(np.float64)  # [M, B, 48]
    posp = outs[:, :, NCAND : NCAND + NCAMS].astype(np.float64)
    sexp = outs[:, :, NCAND + NCAMS :].astype(np.float64)

    # device stores sum((sims+pm)*pm) = 16 - 4*sims_pos per (core, block)
    pos_vals = (MASK_VAL * MASK_VAL - posp.sum(axis=0)) / (-MASK_VAL)
    s_block = sexp.sum(axis=0)  # [B, 6] sum(exp(20*sims)) per camera block
    lse_block = np.log(s_block)  # logsumexp of own-camera logits

    # [B, M*NT, 16] per-(core,chunk) candidate lists
    percl = (
        cand.transpose(1, 0, 2)
        .reshape(B, M, NT, KITC * 8)
        .reshape(B, M * NT, KITC * 8)
    )
    flat = percl.reshape(B, -1)
    top50 = -np.partition(-flat, BG_KNN - 1, axis=1)[:, :BG_KNN]
    t50 = top50[:, BG_KNN - 1]  # [B] 50th largest of the union

    # Exactness certificate: every (core,chunk)'s smallest extracted
    # candidate must be strictly below the union's 50th value, which
    # proves no unseen value can reach the global top-50.
    cmin = percl.min(axis=2)  # [B, M*NT]
    bad = (cmin >= t50[:, None]).any(axis=1)
    if bad.any():
        # Exact fallback for the (astronomically unlikely) insufficient
        # rows: recompute their full similarity row on the host.
        FALLBACK_COUNT += int(bad.sum())
        assert features is not None and memory is not None
        mem_flat = np.asarray(memory, np.float32).reshape(NG, D)
        lab = np.asarray(labels).astype(np.int64)
        idx = np.nonzero(bad)[0]
        sims = (
            np.asarray(features, np.float32)[idx] @ mem_flat.T
        )  # [nbad, NG]
        cols = np.arange(NG)
        for pos, i in enumerate(idx):
            row = sims[pos].copy()
            row[cols % C == lab[i]] = MASK_VAL
            top50[i] = -np.sort(-row)[:BG_KNN]

    return np.float32(_loss_from_parts(pos_vals, lse_block, top50, cams))


def kernel(features, memory, cams, labels, trace: bool = None):
    global LAST_EXEC_NS
    _install_axon_ntff_hook()
    from concourse.bass_utils import run_bass_kernel_spmd

    features = np.asarray(features, dtype=np.float32)
    memory = np.asarray(memory, dtype=np.float32)
    cams = np.asarray(cams).astype(np.int64)
    labels = np.asarray(labels).astype(np.int64)

    nc = get_nc()

    mem_flat = memory.reshape(NG, D)
    featsT = make_in_maps(features, labels)
    in_maps = []
    for k in range(M):
        cols = shard_cols(k)
        in_maps.append(
            {
                "featsT": featsT,
                "memT": pack_memT(mem_flat, cols),
                "labF": lab_adj(labels, k),
            }
        )

    if trace is None:
        trace = os.environ.get("CAP_TRACE", "1") == "1"
    res = run_bass_kernel_spmd(
        nc, in_maps, core_ids=list(range(M)), trace=trace
    )
    if res.exec_time_ns is not None:
        LAST_EXEC_NS = res.exec_time_ns

    outs = np.stack([r["out"] for r in res.results])  # [M, B, OUTC]
    return np.asarray(
        host_combine(outs, cams, features, memory, labels), dtype=np.float32
    )


# ------------------------------------------------------------------ helpers
def expected_core_out(features, memory, labels, k: int) -> np.ndarray:
    """Numpy model of what core k's device program should output [B, OUTC]."""
    mem_flat = np.asarray(memory, np.float32).reshape(NG, D)
    cols = shard_cols(k)
    sims = np.asarray(features, np.float32) @ mem_flat[cols].T  # [B, NL]
    lab = np.asarray(labels).astype(np.int64)
    pmask = (cols % C)[None, :] == lab[:, None]  # [B, NL]
    out = np.zeros((B, OUTC), np.float32)
    maskedv = sims + (pmask * np.float32(MASK_VAL)).astype(np.float32)
    for j in range(NCAMS):
        jsl = slice(j * W, (j + 1) * W)
        out[:, NCAND + j] = (
            maskedv[:, jsl] * (pmask[:, jsl] * np.float32(MASK_VAL))
        ).sum(axis=1)
        out[:, NCAND + NCAMS + j] = np.exp(
            INV_BETA * sims[:, jsl].astype(np.float64)
        ).sum(axis=1)
    for nt in range(NT):
        chunk = maskedv[:, nt * NCHUNK : (nt + 1) * NCHUNK]
        srt = -np.sort(-chunk, axis=1)
        out[:, nt * KITC * 8 : (nt + 1) * KITC * 8] = srt[:, : KITC * 8]
    return out


# revision 35
# speedup vs baseline: 1.2473x; 1.1769x over previous
"""Distributed CAP-memory loss kernel for 8 TRN2 NeuronCores.

Problem (see reference): given unit-norm features [B=256, D=2048] and a
memory bank [6, 2000, 2048], compute
  loss = sum_cam mean_cam(per-camera proxy CE)
       + 0.5 * sum_cam mean_cam(assoc loss over 6 positives + 50 hard negatives)

Distribution strategy (column/class sharding, interleaved):
  The 12000 memory rows are split so core k owns columns
  {j*2000 + k*250 + r : j in [0,6), r in [0,250)} -- i.e. an identical
  250-wide slice of every camera block.  All 8 cores therefore run the
  exact same program (true SPMD) on 1500 columns each:
    * sims_local = feats @ memT_local            (PE, f32[r])
    * per-camera-block partial sum(exp(20*sims)) (ACT, accum)
    * positive extraction via iota==label mask   (DVE)
    * top-16 of each positive-masked 500-col     (DVE max8 + match_replace)
      chunk -> 48 candidates per core, 384 per row globally
  The host merges the per-core stats ([256, 60] each): global top-50
  from the 384 candidates (with an exactness certificate -- see
  host_combine -- and an exact fallback), log-sum-exp combines, segment
  sums -> scalar loss.

Inputs are pre-transposed on the host so every matmul operand is
K(=D)-major and no on-device transpose is needed.
"""

import os
import sys
import types

import numpy as np

# ---------------------------------------------------------------- constants
B = 256          # batch
D = 2048         # feature dim
NCAMS = 6
C = 2000         # classes per camera
NG = NCAMS * C   # 12000 global columns
M = 8            # cores
W = C // M       # 250: per-core slice width inside each camera block
NL = NCAMS * W   # 1500 local columns per core
P = 128          # partitions
KO = D // P      # 16 contraction chunks
BT = B // P      # 2 batch tiles
NT = 3           # matmul column chunks
NCHUNK = NL // NT  # 500
BETA = 0.05
INV_BETA = 1.0 / BETA  # 20.0
BG_KNN = 50
KITC = 2         # top-8 iterations per 500-col chunk -> 16 cand/chunk
NCAND = KITC * 8 * NT  # 48 candidates per core
# Mask value -4: positives become sims-4 in [-5,-3], below every genuine
# cosine sim (>= -1), so they never reach a top-16.  Multiplying the masked
# value by -4 (exact in fp) gives 16 - 4*sims at the positive and 0
# elsewhere, so the host recovers the positive sim as (16 - sum)/4.
MASK_VAL = -4.0
REPL_VAL = -30000.0
OUTC = NCAND + 2 * NCAMS  # 48 topk | 6 pos | 6 sumexp

LAST_EXEC_NS = None
FALLBACK_COUNT = 0
_NC_CACHE = {}


def _install_axon_ntff_hook():
    """The agent image's antenv lacks axon_hooks; synthesize it so
    run_bass_kernel_spmd(trace=True) can capture NTFF profiles."""
    if "antenv.axon_hooks" in sys.modules:
        return
    mod = types.ModuleType("antenv.axon_hooks")
    state = {"hook": None}
    mod.set_axon_ntff_profile_hook = lambda h: state.__setitem__("hook", h)
    mod.get_axon_ntff_profile_hook = lambda: state["hook"]
    sys.modules["antenv.axon_hooks"] = mod
    try:
        import antenv

        antenv.axon_hooks = mod
    except Exception:
        pass
    try:
        from trn_agent_boot.trn_boot import _ntff_profile_via_ctypes

        hook = _ntff_profile_via_ctypes("/opt/axon/libaxon_pjrt.so")
        if hook is not None:
            mod.set_axon_ntff_profile_hook(hook)
    except Exception:
        pass


def build_nc(mm_dtype_name: str = "float32r"):
    """Build + compile the single SPMD Bass program shared by all 8 cores."""
    import concourse.bacc as bacc
    import concourse.mybir as mybir
    import concourse.tile as tile

    f32 = mybir.dt.float32
    mm_dt = getattr(mybir.dt, mm_dtype_name)
    A = mybir.AluOpType
    AF = mybir.ActivationFunctionType

    nc = bacc.Bacc(
        "TRN2",
        target_bir_lowering=False,
        debug=False,
        enable_asserts=False,
        num_devices=M,
    )

    # Host-packed layouts: one fully-contiguous row per SBUF partition.
    featsT_d = nc.dram_tensor("featsT", [P, KO * B], mm_dt, kind="ExternalInput")
    memT_d = nc.dram_tensor("memT", [P, KO * NL], mm_dt, kind="ExternalInput")
    # labAdj = label - 250*core_id: the on-device iota holds (col % 250), so
    # equality against labAdj marks exactly this core's positive columns.
    lab_d = nc.dram_tensor("labF", [B, 1], f32, kind="ExternalInput")
    out_d = nc.dram_tensor("out", [B, OUTC], f32, kind="ExternalOutput")

    lab_r = lab_d.rearrange("(bt p) o -> p (bt o)", p=P)

    with tile.TileContext(nc) as tc:
        with (
            tc.tile_pool(name="big", bufs=1) as big,
            tc.tile_pool(name="work", bufs=BT) as work,
            tc.tile_pool(name="scr", bufs=4) as scr,
            tc.tile_pool(name="psum", bufs=BT * NT, space="PSUM") as psum,
        ):
            # Stream inputs in PE-consumption order (ko-major), round-robin
            # across the three DMA-capable queues (~130 GB/s each, ~360
            # aggregate) so the matmul pipeline starts within a few us.
            featsT_sb = big.tile([P, KO * B], mm_dt)
            memT_sb = big.tile([P, KO * NL], mm_dt)
            queues = [nc.sync, nc.scalar, nc.gpsimd]
            for nt in range(NT):
                for ko in range(KO):
                    q = queues[ko % 3]
                    if nt == 0:
                        fsl = slice(ko * B, (ko + 1) * B)
                        q.dma_start(featsT_sb[:, fsl], featsT_d[:, fsl])
                    msl = slice(
                        ko * NL + nt * NCHUNK, ko * NL + (nt + 1) * NCHUNK
                    )
                    q.dma_start(memT_sb[:, msl], memT_d[:, msl])
            lab_sb = big.tile([P, BT], f32)
            nc.gpsimd.dma_start(lab_sb[:], lab_r)
            iota_sb = big.tile([P, NL], f32)
            nc.gpsimd.iota(
                iota_sb[:].rearrange("p (j w) -> p j w", w=W),
                pattern=[[0, NCAMS], [1, W]],
                base=0,
                channel_multiplier=0,
                allow_small_or_imprecise_dtypes=True,
            )

            pm = [
                work.tile([P, NL], f32, tag="pm", name=f"pm{b}")
                for b in range(BT)
            ]
            masked = [
                work.tile([P, NL], f32, tag="masked", name=f"masked{b}")
                for b in range(BT)
            ]
            outs = [
                work.tile([P, OUTC], f32, tag="outs", name=f"outs{b}")
                for b in range(BT)
            ]
            tmp = [
                work.tile([P, NL], f32, tag="tmp", name=f"tmp{b}")
                for b in range(BT)
            ]

            # positive mask: -20000 where (global_col % 2000) == label, else 0
            for bt in range(BT):
                nc.vector.tensor_scalar(
                    pm[bt][:],
                    iota_sb[:],
                    lab_sb[:, bt : bt + 1],
                    MASK_VAL,
                    op0=A.is_equal,
                    op1=A.mult,
                )

            pstiles = [
                psum.tile([P, NCHUNK], f32, tag="ps", name=f"ps{b}_{n}")
                for b in range(BT)
                for n in range(NT)
            ]

            def mm(bt, nt, ko):
                nc.tensor.matmul(
                    pstiles[bt * NT + nt][:],
                    featsT_sb[:, ko * B + bt * P : ko * B + (bt + 1) * P],
                    memT_sb[:, ko * NL + nt * NCHUNK : ko * NL + (nt + 1) * NCHUNK],
                    start=(ko == 0),
                    stop=(ko == KO - 1),
                )

            def epilogue(bt, nt):
                ps = pstiles[bt * NT + nt]
                nsl = slice(nt * NCHUNK, (nt + 1) * NCHUNK)
                # masked sims in one pass: sims + pm
                nc.vector.tensor_tensor(
                    out=masked[bt][:, nsl],
                    in0=ps[:],
                    in1=pm[bt][:, nsl],
                    op=A.add,
                )
                for s in range(NCHUNK // W):
                    j = nt * (NCHUNK // W) + s
                    pse = ps[:, s * W : (s + 1) * W]
                    # per-camera-block sum(exp(sims/beta)); sims in (-1,1)
                    # so exp(20*sims) stays in f32 range without bias
                    et = scr.tile([P, W], f32, tag="exp")
                    nc.scalar.activation(
                        et[:],
                        pse,
                        AF.Exp,
                        scale=INV_BETA,
                        accum_out=outs[bt][
                            :, NCAND + NCAMS + j : NCAND + NCAMS + j + 1
                        ],
                    )
                # positive extraction: (sims+pm)*pm = 16-4*sims at the
                # positive slot, 0 elsewhere (gpsimd: SBUF-only inputs)
                nc.gpsimd.tensor_tensor(
                    out=tmp[bt][:, nsl],
                    in0=masked[bt][:, nsl],
                    in1=pm[bt][:, nsl],
                    op=A.mult,
                )
                nc.vector.reduce_sum(
                    out=outs[bt][:, NCAND + 2 * nt : NCAND + 2 * nt + 2],
                    in_=tmp[bt][:, nsl].rearrange("p (j w) -> p j w", w=W),
                    axis=mybir.AxisListType.X,
                )
                # top-16 of this 500-col chunk
                for it in range(KITC):
                    col = (nt * KITC + it) * 8
                    nc.vector.max(
                        out=outs[bt][:, col : col + 8],
                        in_=masked[bt][:, nsl],
                    )
                    if it < KITC - 1:
                        nc.vector.match_replace(
                            out=masked[bt][:, nsl],
                            in_to_replace=outs[bt][:, col : col + 8],
                            in_values=masked[bt][:, nsl],
                            imm_value=REPL_VAL,
                        )

            for nt in range(NT):
                for bt in range(BT):
                    for ko in range(KO):
                        mm(bt, nt, ko)
                    epilogue(bt, nt)

            for bt in range(BT):
                nc.sync.dma_start(out_d[bt * P : (bt + 1) * P, :], outs[bt][:])

    nc.compile()
    return nc


def get_nc(mm_dtype_name: str = None):
    if mm_dtype_name is None:
        mm_dtype_name = os.environ.get("CAP_MM_DTYPE", "bfloat16")
    if mm_dtype_name not in _NC_CACHE:
        _NC_CACHE[mm_dtype_name] = build_nc(mm_dtype_name)
    return _NC_CACHE[mm_dtype_name]


def shard_cols(k: int) -> np.ndarray:
    """Global memory-bank columns owned by core k."""
    return (
        np.arange(NCAMS)[:, None] * C + k * W + np.arange(W)[None, :]
    ).reshape(-1)


def _mm_np_dtype():
    name = os.environ.get("CAP_MM_DTYPE", "bfloat16")
    if name == "bfloat16":
        import ml_dtypes

        return np.dtype(ml_dtypes.bfloat16)
    return np.dtype(np.float32)


def pack_featsT(features: np.ndarray) -> np.ndarray:
    """[B, D] -> [P, KO*B] with row p holding feats.T[ko*128+p, :] runs."""
    arr = features.T.reshape(KO, P, B).transpose(1, 0, 2).reshape(P, KO * B)
    return np.ascontiguousarray(arr).astype(_mm_np_dtype())


def pack_memT(mem_flat: np.ndarray, cols: np.ndarray) -> np.ndarray:
    """[NG, D] -> [P, KO*NL] packed like pack_featsT for this core's cols."""
    arr = (
        mem_flat[cols].T.reshape(KO, P, NL).transpose(1, 0, 2).reshape(P, KO * NL)
    )
    return np.ascontiguousarray(arr).astype(_mm_np_dtype())


def make_in_maps(features: np.ndarray, labels: np.ndarray):
    featsT = pack_featsT(features)
    return featsT


def lab_adj(labels: np.ndarray, k: int) -> np.ndarray:
    return (labels.astype(np.float32) - np.float32(W * k)).reshape(B, 1)


def _loss_from_parts(pos_vals, lse_block, top50, cams):
    rows = np.arange(B)
    ce = lse_block[rows, cams] - INV_BETA * pos_vals[rows, cams]
    logits = np.concatenate([INV_BETA * pos_vals, INV_BETA * top50], axis=1)
    mx = logits.max(axis=1, keepdims=True)
    lse56 = mx[:, 0] + np.log(np.exp(logits - mx).sum(axis=1))
    assoc = lse56 - (INV_BETA / NCAMS) * pos_vals.sum(axis=1)

    counts = np.bincount(cams, minlength=NCAMS).astype(np.float64)
    ce_sum = np.bincount(cams, weights=ce, minlength=NCAMS)
    as_sum = np.bincount(cams, weights=assoc, minlength=NCAMS)
    safe = np.maximum(counts, 1.0)
    present = counts > 0
    return np.sum(np.where(present, ce_sum / safe, 0.0)) + np.sum(
        np.where(present, 0.5 * as_sum / safe, 0.0)
    )


def host_combine(outs, cams, features=None, memory=None, labels=None):
    """outs: [M, B, OUTC] device results; cams: [B] int."""
    global FALLBACK_COUNT
    cand = outs[:, :, :NCAND].astype(np.float64)  # [M, B, 48]
    posp = outs[:, :, NCAND : NCAND + NCAMS].astype(np.float64)
    sexp = outs[:, :, NCAND + NCAMS :].astype(np.float64)

    # device stores sum((sims+pm)*pm) = 16 - 4*sims_pos per (core, block)
    pos_vals = (MASK_VAL * MASK_VAL - posp.sum(axis=0)) / (-MASK_VAL)
    s_block = sexp.sum(axis=0)  # [B, 6] sum(exp(20*sims)) per camera block
    lse_block = np.log(s_block)  # logsumexp of own-camera logits

    # [B, M*NT, 16] per-(core,chunk) candidate lists
    percl = (
        cand.transpose(1, 0, 2)
        .reshape(B, M, NT, KITC * 8)
        .reshape(B, M * NT, KITC * 8)
    )
    flat = percl.reshape(B, -1)
    top50 = -np.partition(-flat, BG_KNN - 1, axis=1)[:, :BG_KNN]
    t50 = top50[:, BG_KNN - 1]  # [B] 50th largest of the union

    # Exactness certificate: every (core,chunk)'s smallest extracted
    # candidate must be strictly below the union's 50th value, which
    # proves no unseen value can reach the global top-50.
    cmin = percl.min(axis=2)  # [B, M*NT]
    bad = (cmin >= t50[:, None]).any(axis=1)
    if bad.any():
        # Exact fallback for the (astronomically unlikely) insufficient
        # rows: recompute their full similarity row on the host.
        FALLBACK_COUNT += int(bad.sum())
        assert features is not None and memory is not None
        mem_flat = np.asarray(memory, np.float32).reshape(NG, D)
        lab = np.asarray(labels).astype(np.int64)
        idx = np.nonzero(bad)[0]
        sims = (
            np.asarray(features, np.float32)[idx] @ mem_flat.T
        )  # [nbad, NG]
        cols = np.arange(NG)
        for pos, i in enumerate(idx):
            row = sims[pos].copy()
            row[cols % C == lab[i]] = MASK_VAL
            top50[i] = -np.sort(-row)[:BG_KNN]

    return np.float32(_loss_from_parts(pos_vals, lse_block, top50, cams))


def kernel(features, memory, cams, labels, trace: bool = None):
    global LAST_EXEC_NS
    _install_axon_ntff_hook()
    from concourse.bass_utils import run_bass_kernel_spmd

    features = np.asarray(features, dtype=np.float32)
    memory = np.asarray(memory, dtype=np.float32)
    cams = np.asarray(cams).astype(np.int64)
    labels = np.asarray(labels).astype(np.int64)

    nc = get_nc()

    mem_flat = memory.reshape(NG, D)
    featsT = make_in_maps(features, labels)
    in_maps = []
    for k in range(M):
        cols = shard_cols(k)
        in_maps.append(
            {
                "featsT": featsT,
                "memT": pack_memT(mem_flat, cols),
                "labF": lab_adj(labels, k),
            }
        )

    if trace is None:
        trace = os.environ.get("CAP_TRACE", "1") == "1"
    res = run_bass_kernel_spmd(
        nc, in_maps, core_ids=list(range(M)), trace=trace
    )
    if res.exec_time_ns is not None:
        LAST_EXEC_NS = res.exec_time_ns

    outs = np.stack([r["out"] for r in res.results])  # [M, B, OUTC]
    return np.asarray(
        host_combine(outs, cams, features, memory, labels), dtype=np.float32
    )


# ------------------------------------------------------------------ helpers
def expected_core_out(features, memory, labels, k: int) -> np.ndarray:
    """Numpy model of what core k's device program should output [B, OUTC]."""
    mem_flat = np.asarray(memory, np.float32).reshape(NG, D)
    cols = shard_cols(k)
    sims = np.asarray(features, np.float32) @ mem_flat[cols].T  # [B, NL]
    lab = np.asarray(labels).astype(np.int64)
    pmask = (cols % C)[None, :] == lab[:, None]  # [B, NL]
    out = np.zeros((B, OUTC), np.float32)
    maskedv = sims + (pmask * np.float32(MASK_VAL)).astype(np.float32)
    for j in range(NCAMS):
        jsl = slice(j * W, (j + 1) * W)
        out[:, NCAND + j] = (
            maskedv[:, jsl] * (pmask[:, jsl] * np.float32(MASK_VAL))
        ).sum(axis=1)
        out[:, NCAND + NCAMS + j] = np.exp(
            INV_BETA * sims[:, jsl].astype(np.float64)
        ).sum(axis=1)
    for nt in range(NT):
        chunk = maskedv[:, nt * NCHUNK : (nt + 1) * NCHUNK]
        srt = -np.sort(-chunk, axis=1)
        out[:, nt * KITC * 8 : (nt + 1) * KITC * 8] = srt[:, : KITC * 8]
    return out
